# revision 20
# baseline (speedup 1.0000x reference)
"""MHA kernel for TRN2: x[8,512,32,32], 8 heads, S=1024, C=512.

Sharding: data-parallel over batch N=8 -> one batch item per NeuronCore.
Per-core layout (all transpose-free, bf16 matmuls, fp32 psum):
  qkT[e,s]  = w_qkvT.T @ x            (e on partitions; w cols host-reordered
                                       [q_p0|k_p0|...|q_p3|k_p3|v])
  v[s,e]    = x.T @ w_v               (s on partitions)
  scoresT   = kT_h.T @ qT_h           (k_s on partitions; head pair packed at
                                       PE rows 0-63 / 64-127, dual-issued)
  P         = exp(scoresT/8)          (ACT, 1024-wide from PSUM)
  oT_aug    = [v_h | 1].T @ P         (M=65; row 64 = softmax denominator r)
  oT        = oT_aug[:64] * (1/r)     (psum copied to sbuf to free the PV
                                       accumulator early; recip + gpsimd
                                       partition_broadcast + mul)
  yT[o,s]   = w_outT.T @ oT           (b_out added host-side; y stored bf16)
Schedule: per head-pair step, QK->exp->PV fused; PVs catch up over slots 4-7;
pair p's PV(7) + normalization run in step p+1's slot 0; qkv groups for pair
p+1 drain inside step p; tail does partial out-projection during the last
normalization chain.
"""

import numpy as np
import ml_dtypes

import concourse.bacc as bacc
import concourse.mybir as mybir
import concourse.tile as tile
from concourse.bass_utils import run_bass_kernel_spmd

P = 128
S = 1024          # sequence = 32*32
C = 512           # channels
NH = 8            # heads
HD = 64           # head dim
CT = C // P       # 4 c-tiles
MT = S // P       # 8 s-tiles
NP = NH // 2      # 4 head pairs
BF = mybir.dt.bfloat16
F32 = mybir.dt.float32
DRAIN = ((0, 0), (1, 0), (0, 1), (1, 1))  # (hh, nt)
PV_AT = {4: (0,), 5: (1, 2), 6: (3, 4), 7: (5, 6)}  # PV(7) -> next step

_cache = {}


def build_program():
    nc = bacc.Bacc("TRN2", target_bir_lowering=False, debug=False, num_devices=8)
    x_d = nc.dram_tensor("x", [C, S], BF, kind="ExternalInput").ap()
    # host-reordered columns: [q_p0|k_p0|q_p1|k_p1|q_p2|k_p2|q_p3|k_p3|v]
    wq_d = nc.dram_tensor("wq", [C, 3 * C], BF, kind="ExternalInput").ap()
    wo_d = nc.dram_tensor("wo", [C, C], BF, kind="ExternalInput").ap()
    y_d = nc.dram_tensor("y", [C, S], BF, kind="ExternalOutput").ap()

    with tile.TileContext(nc) as tc:
        with (
            tc.tile_pool(name="const", bufs=1) as cpool,
            tc.tile_pool(name="qk", bufs=1) as qkpool,
            tc.tile_pool(name="vp", bufs=1) as vpool,
            tc.tile_pool(name="pp", bufs=10) as ppool,
            tc.tile_pool(name="ot", bufs=1) as opool,
            tc.tile_pool(name="yp", bufs=8) as ypool,
            tc.tile_pool(name="cp", bufs=5) as cppool,
            tc.tile_pool(name="misc", bufs=4) as mpool,
            tc.tile_pool(name="psq", bufs=2, space="PSUM") as psq_pool,
            tc.tile_pool(name="pso", bufs=4, space="PSUM") as pso_pool,
        ):
            # ---- load inputs; per-ct x tiles; serial sync triggers give a
            # natural stagger that keeps the early DMA rails on x ----
            wp0_sb = cpool.tile([P, CT * 256], BF, name="wp0", tag="wp0")
            x_sb = [cpool.tile([P, S], BF, name=f"x{ct}", tag=f"x{ct}")
                    for ct in range(CT)]
            wvv_sb = cpool.tile([P, CT * 512], BF, name="wvv", tag="wvv")
            wpr_sb = cpool.tile([P, CT * 768], BF, name="wpr", tag="wpr")
            wo_sb = cpool.tile([P, CT * C], BF, name="wosb", tag="wosb")
            wqv = wq_d.rearrange("(f p) e -> p f e", p=P)
            nc.sync.dma_start(
                wp0_sb.rearrange("p (f e) -> p f e", f=CT), wqv[:, :, 0:256]
            )
            for ct in range(CT):
                nc.sync.dma_start(x_sb[ct][:], x_d[ct * P:(ct + 1) * P, :])
            nc.sync.dma_start(
                wvv_sb.rearrange("p (f e) -> p f e", f=CT), wqv[:, :, 1024:1536]
            )
            nc.sync.dma_start(
                wpr_sb.rearrange("p (f e) -> p f e", f=CT), wqv[:, :, 256:1024]
            )
            nc.sync.dma_start(
                wo_sb.rearrange("p (f e) -> p f e", f=CT),
                wo_d.rearrange("(f p) e -> p f e", p=P),
            )

            def x_slice(ct, lo, hi):
                return x_sb[ct][:, lo:hi]

            def w_slice(col, ct):
                if col < 256:
                    return wp0_sb[:, ct * 256 + col:ct * 256 + col + 128]
                if col < 1024:
                    c = col - 256
                    return wpr_sb[:, ct * 768 + c:ct * 768 + c + 128]
                c = col - 1024
                return wvv_sb[:, ct * 512 + c:ct * 512 + c + 512]

            q_sb = [qkpool.tile([P, S], BF, name=f"q{p}", tag=f"q{p}")
                    for p in range(NP)]
            k_sb = [qkpool.tile([P, S], BF, name=f"k{p}", tag=f"k{p}")
                    for p in range(NP)]
            v_sb = [None] * MT
            oT_sb = [opool.tile([P, S], BF, name=f"o{ct}", tag=f"o{ct}")
                     for ct in range(CT)]

            def g_qkv(dst, col, nt, pool, copy_eng=None):
                ps = pool.tile([P, 512], F32, name="gq", tag=pool.name)
                for ct in range(CT):
                    nc.tensor.matmul(
                        ps[:],
                        w_slice(col, ct),
                        x_slice(ct, nt * 512, (nt + 1) * 512),
                        start=(ct == 0), stop=(ct == CT - 1),
                    )
                dst_sl = dst[:, nt * 512:(nt + 1) * 512]
                if copy_eng == "scalar":
                    nc.scalar.copy(dst_sl, ps[:])
                else:
                    nc.vector.tensor_copy(dst_sl, ps[:])

            def g_v(mt, pool):
                ps = pool.tile([P, 512], F32, name="gv", tag=pool.name)
                for ct in range(CT):
                    nc.tensor.matmul(
                        ps[:],
                        x_slice(ct, mt * P, (mt + 1) * P),
                        w_slice(1024, ct),
                        start=(ct == 0), stop=(ct == CT - 1),
                    )
                vt = vpool.tile([P, NH * (HD + 1)], BF, name=f"v{mt}", tag=f"v{mt}")
                vv = vt.rearrange("p (h e) -> p h e", e=HD + 1)
                nc.gpsimd.memset(vv[:, :, HD:HD + 1], 1.0)
                nc.vector.tensor_copy(
                    vv[:, :, 0:HD], ps.rearrange("p (h d) -> p h d", d=HD)
                )
                v_sb[mt] = vt

            def qk_groups(p):
                return [("q", p, 0), ("k", p, 0), ("q", p, 1), ("k", p, 1)]

            drains = {
                0: [("v", m, 0) for m in range(MT)] + qk_groups(1),
                1: qk_groups(2),
                2: qk_groups(3),
                3: [],
            }
            drain_quota = {
                0: [0, 1, 2, 2, 2, 2, 2, 1],
                1: [1, 1, 1, 1, 0, 0, 0, 0],
                2: [1, 1, 1, 1, 0, 0, 0, 0],
                3: [0] * 8,
            }

            def emit_drain(item):
                kind, a, nt = item
                if kind == "v":
                    g_v(a, psq_pool)
                elif kind == "q":
                    g_qkv(q_sb[a], a * 256, nt, psq_pool)
                else:
                    g_qkv(k_sb[a], a * 256 + 128, nt, psq_pool)

            def emit_qk_half(p, s, nt, ptiles):
                psq = psq_pool.tile([P, 1024], F32, name="psq", tag="psq")
                for hh in range(2):
                    nc.tensor.matmul(
                        psq[:, hh * 512:(hh + 1) * 512],
                        k_sb[p][hh * HD:(hh + 1) * HD, s * P:(s + 1) * P],
                        q_sb[p][hh * HD:(hh + 1) * HD, nt * 512:(nt + 1) * 512],
                        start=True, stop=True,
                    )
                pt = ppool.tile([P, 1024], BF, name="ptile", tag="ptile")
                nc.scalar.activation(
                    pt[:], psq[:], mybir.ActivationFunctionType.Exp,
                    scale=float(1.0 / np.sqrt(HD)),
                )
                ptiles[(s, nt)] = pt

            def emit_pv(pp, pso_t, ptiles, m):
                for idx, (hh, nt) in enumerate(DRAIN):
                    h = 2 * pp + hh
                    nc.tensor.matmul(
                        pso_t[idx][0:HD + 1, :],
                        v_sb[m][:, h * (HD + 1):(h + 1) * (HD + 1)],
                        ptiles[(m, nt)][:, hh * 512:(hh + 1) * 512],
                        start=(m == 0), stop=(m == MT - 1),
                    )

            def emit_chain_pre(pso_t):
                """recip+broadcast of the softmax denominators (cheap vector
                ops, no head-of-line risk)."""
                bc = []
                for idx in range(4):
                    r0 = mpool.tile([1, 512], F32, name="rr", tag="rr")
                    nc.vector.tensor_copy(r0[0:1, :], pso_t[idx][HD:HD + 1, :])
                    r1 = mpool.tile([1, 512], F32, name="ri", tag="ri")
                    nc.vector.reciprocal_approx_fast(r1[0:1, :], r0[0:1, :])
                    b0 = mpool.tile([HD, 512], F32, name="bc", tag="bc")
                    nc.gpsimd.partition_broadcast(b0[:], r1[0:1, :], channels=HD)
                    bc.append(b0)
                return bc

            def emit_chain_muls(pp, pso_t, bc):
                for idx, (hh, nt) in enumerate(DRAIN):
                    nc.vector.tensor_mul(
                        oT_sb[pp][hh * HD:(hh + 1) * HD, nt * 512:(nt + 1) * 512],
                        pso_t[idx][0:HD, :], bc[idx][:],
                    )

            def emit_tail_chain(pso_t, idxs):
                """rr+recip (vector), bcast (gpsimd), then muls — ordered to
                avoid vector head-of-line blocking."""
                bc = {}
                for idx in idxs:
                    r0 = mpool.tile([1, 512], F32, name="rr", tag="rr")
                    nc.vector.tensor_copy(r0[0:1, :], pso_t[idx][HD:HD + 1, :])
                    r1 = mpool.tile([1, 512], F32, name="ri", tag="ri")
                    nc.vector.reciprocal_approx_fast(r1[0:1, :], r0[0:1, :])
                    b0 = mpool.tile([HD, 512], F32, name="bc", tag="bc")
                    nc.gpsimd.partition_broadcast(b0[:], r1[0:1, :], channels=HD)
                    bc[idx] = b0
                for idx in idxs:
                    hh, nt = DRAIN[idx]
                    nc.vector.tensor_mul(
                        oT_sb[3][hh * HD:(hh + 1) * HD, nt * 512:(nt + 1) * 512],
                        pso_t[idx][0:HD, :], bc[idx][:],
                    )

            def proj_group(g, st, pool, copy_eng="scalar"):
                ps = pool.tile([P, 512], F32, name="op", tag=pool.name)
                for ct in range(CT):
                    nc.tensor.matmul(
                        ps[:],
                        wo_sb[:, ct * 512 + g * P:ct * 512 + (g + 1) * P],
                        oT_sb[ct][:, st * 512:(st + 1) * 512],
                        start=(ct == 0), stop=(ct == CT - 1),
                    )
                yt = ypool.tile([P, 512], BF, name="yt", tag="yt")
                if copy_eng == "scalar":
                    nc.scalar.activation(
                        yt[:], ps[:], mybir.ActivationFunctionType.Copy
                    )
                else:
                    nc.vector.tensor_copy(yt[:], ps[:])
                nc.sync.dma_start(
                    y_d[g * P:(g + 1) * P, st * 512:(st + 1) * 512], yt[:]
                )

            # ---- attention steps ----
            prev = None  # (p-1, pso_t, ptiles)
            for p in range(3):
                pso_t = [pso_pool.tile([P, 512], F32, name=f"pso{i}", tag="pso")
                         for i in range(4)]
                ptiles = {}
                dq = list(drains[p])
                quota = drain_quota[p]
                s_start = 0
                if p == 0:
                    # minimal pre-phase: just enough to start the exp chain
                    g_qkv(q_sb[0], 0, 0, pso_pool)
                    g_qkv(k_sb[0], 128, 0, pso_pool, copy_eng="scalar")
                    emit_qk_half(0, 0, 0, ptiles)
                    g_qkv(q_sb[0], 0, 1, pso_pool)
                    g_qkv(k_sb[0], 128, 1, pso_pool, copy_eng="scalar")
                    emit_qk_half(0, 0, 1, ptiles)
                    s_start = 1
                prev_bc = None
                for s in range(s_start, MT):
                    emit_qk_half(p, s, 0, ptiles)
                    emit_qk_half(p, s, 1, ptiles)
                    if prev is not None and s == 0:
                        emit_pv(prev[0], prev[1], prev[2], 7)
                    for m in PV_AT.get(s, ()):
                        emit_pv(p, pso_t, ptiles, m)
                    for _ in range(quota[s]):
                        if dq:
                            emit_drain(dq.pop(0))
                    if prev is not None and s == 0:
                        prev_bc = emit_chain_pre(prev[1])
                    if prev is not None and s == 3:
                        emit_chain_muls(prev[0], prev[1], prev_bc)
                prev = (p, pso_t, ptiles)

            # ---- last step (p=3): nt-split so the st0 half of the output
            # projection completes during the nt1 exp phase; st0 groups 0/1
            # run as ct0-2 partials in the nt0 phase (PSUM slots for the nt1
            # accumulators are still free there, and it keeps the PE warm) ----
            ptiles = {}
            pso_t = [None] * 4
            pso_t[0] = pso_pool.tile([P, 512], F32, name="pso0", tag="pso")
            pso_t[1] = pso_pool.tile([P, 512], F32, name="pso1", tag="pso")
            pgA = [pso_pool.tile([P, 512], F32, name=f"pga{g}", tag="pso")
                   for g in range(2)]

            def emit_pv3(m, idxs):
                for idx in idxs:
                    hh, nt = DRAIN[idx]
                    h = 6 + hh
                    nc.tensor.matmul(
                        pso_t[idx][0:HD + 1, :],
                        v_sb[m][:, h * (HD + 1):(h + 1) * (HD + 1)],
                        ptiles[(m, nt)][:, hh * 512:(hh + 1) * 512],
                        start=(m == 0), stop=(m == MT - 1),
                    )

            PV3_AT = {3: (0,), 4: (1,), 5: (2, 3), 6: (4, 5), 7: (6,)}
            prev_bc = None
            for s in range(MT):  # nt0 phase
                emit_qk_half(3, s, 0, ptiles)
                if s == 0:
                    emit_pv(prev[0], prev[1], prev[2], 7)
                for m in PV3_AT.get(s, ()):
                    emit_pv3(m, (0, 1))
                if s == 0:
                    prev_bc = emit_chain_pre(prev[1])
                if s == 2:
                    emit_chain_muls(prev[0], prev[1], prev_bc)
                if s in (5, 6):
                    g = s - 5
                    for ct in range(3):
                        nc.tensor.matmul(
                            pgA[g][:],
                            wo_sb[:, ct * 512 + g * P:ct * 512 + (g + 1) * P],
                            oT_sb[ct][:, 0:512],
                            start=(ct == 0), stop=False,
                        )
            pso_t[2] = pso_pool.tile([P, 512], F32, name="pso2", tag="pso")
            pso_t[3] = pso_pool.tile([P, 512], F32, name="pso3", tag="pso")
            for s in range(MT):  # nt1 phase
                emit_qk_half(3, s, 1, ptiles)
                if s == 0:
                    emit_pv3(7, (0, 1))
                for m in PV3_AT.get(s, ()):
                    emit_pv3(m, (2, 3))
                if s == 1:
                    emit_tail_chain(pso_t, (0, 1))
                if s == 2:
                    # finish st0 groups 0/1 with ct=3, copy + DMA
                    for g in range(2):
                        nc.tensor.matmul(
                            pgA[g][:],
                            wo_sb[:, 3 * 512 + g * P:3 * 512 + (g + 1) * P],
                            oT_sb[3][:, 0:512],
                            start=False, stop=True,
                        )
                        yt = ypool.tile([P, 512], BF, name="yt", tag="yt")
                        nc.vector.tensor_copy(yt[:], pgA[g][:])
                        nc.sync.dma_start(y_d[g * P:(g + 1) * P, 0:512], yt[:])
                if s == 3:
                    proj_group(2, 0, psq_pool, copy_eng="vector")
                if s == 4:
                    proj_group(3, 0, psq_pool, copy_eng="vector")
            # ---- tail: PV(3,7) nt1, chains idx2/3, st1 projection ----
            emit_pv3(7, (2, 3))
            emit_tail_chain(pso_t, (2, 3))
            for g in range(CT):
                proj_group(g, 1, pso_pool)

    nc.compile()
    return nc


def get_program():
    if "nc" not in _cache:
        _cache["nc"] = build_program()
    return _cache["nc"]


_COL_ORDER = np.concatenate(
    [np.r_[p * 128:(p + 1) * 128, 512 + p * 128:512 + (p + 1) * 128]
     for p in range(NP)] + [np.r_[1024:1536]]
)


def kernel(x, w_qkv, w_out, b_out, _trace=False, _tmpdir=None):
    x = np.asarray(x, dtype=np.float32)
    w_qkv = np.asarray(w_qkv, dtype=np.float32)
    w_out = np.asarray(w_out, dtype=np.float32)
    b_out = np.asarray(b_out, dtype=np.float32)
    N = x.shape[0]

    xb = x.reshape(N, C, S).astype(ml_dtypes.bfloat16)
    wqT = np.ascontiguousarray(w_qkv.T[:, _COL_ORDER]).astype(ml_dtypes.bfloat16)
    woT = np.ascontiguousarray(w_out.T).astype(ml_dtypes.bfloat16)

    nc = get_program()
    in_maps = [
        {"x": np.ascontiguousarray(xb[n]), "wq": wqT, "wo": woT}
        for n in range(N)
    ]
    res = run_bass_kernel_spmd(
        nc, in_maps, core_ids=list(range(N)), trace=_trace, tmpdir=_tmpdir
    )
    y = np.stack([res.results[n]["y"] for n in range(N)]).astype(np.float32)
    y = y.reshape(N, C, 32, 32)
    y = y + b_out[None, :, None, None]
    if _trace:
        return y, res
    return y


# revision 21
# speedup vs baseline: 1.0224x; 1.0224x over previous
"""MHA kernel for TRN2: x[8,512,32,32], 8 heads, S=1024, C=512.

Sharding: data-parallel over batch N=8 -> one batch item per NeuronCore.
Per-core layout (all transpose-free, bf16 matmuls, fp32 psum):
  qkT[e,s]  = w_qkvT.T @ x            (e on partitions; w cols host-reordered
                                       [q_p0|k_p0|...|q_p3|k_p3|v])
  v[s,e]    = x.T @ w_v               (s on partitions)
  scoresT   = kT_h.T @ qT_h           (k_s on partitions; head pair packed at
                                       PE rows 0-63 / 64-127, dual-issued)
  P         = exp(scoresT/8)          (ACT, 1024-wide from PSUM)
  oT_aug    = [v_h | 1].T @ P         (M=65; row 64 = softmax denominator r)
  oT        = oT_aug[:64] * (1/r)     (psum copied to sbuf to free the PV
                                       accumulator early; recip + gpsimd
                                       partition_broadcast + mul)
  yT[o,s]   = w_outT.T @ oT           (b_out added host-side; y stored bf16)
Schedule: per head-pair step, QK->exp->PV fused; PVs catch up over slots 4-7;
pair p's PV(7) + normalization run in step p+1's slot 0; qkv groups for pair
p+1 drain inside step p; tail does partial out-projection during the last
normalization chain.
"""

import numpy as np
import ml_dtypes

import concourse.bacc as bacc
import concourse.mybir as mybir
import concourse.tile as tile
from concourse.bass_utils import run_bass_kernel_spmd

P = 128
S = 1024          # sequence = 32*32
C = 512           # channels
NH = 8            # heads
HD = 64           # head dim
CT = C // P       # 4 c-tiles
MT = S // P       # 8 s-tiles
NP = NH // 2      # 4 head pairs
BF = mybir.dt.bfloat16
F32 = mybir.dt.float32
DRAIN = ((0, 0), (1, 0), (0, 1), (1, 1))  # (hh, nt)
PV_AT = {4: (0,), 5: (1, 2), 6: (3, 4), 7: (5, 6)}  # PV(7) -> next step

_cache = {}


def build_program():
    nc = bacc.Bacc("TRN2", target_bir_lowering=False, debug=False, num_devices=8)
    x_d = nc.dram_tensor("x", [C, S], BF, kind="ExternalInput").ap()
    # host-reordered columns: [q_p0|k_p0|q_p1|k_p1|q_p2|k_p2|q_p3|k_p3|v]
    wq_d = nc.dram_tensor("wq", [C, 3 * C], BF, kind="ExternalInput").ap()
    wo_d = nc.dram_tensor("wo", [C, C], BF, kind="ExternalInput").ap()
    y_d = nc.dram_tensor("y", [C, S], BF, kind="ExternalOutput").ap()

    with tile.TileContext(nc) as tc:
        with (
            tc.tile_pool(name="const", bufs=1) as cpool,
            tc.tile_pool(name="qk", bufs=1) as qkpool,
            tc.tile_pool(name="vp", bufs=1) as vpool,
            tc.tile_pool(name="pp", bufs=10) as ppool,
            tc.tile_pool(name="ot", bufs=1) as opool,
            tc.tile_pool(name="yp", bufs=8) as ypool,
            tc.tile_pool(name="cp", bufs=5) as cppool,
            tc.tile_pool(name="misc", bufs=4) as mpool,
            tc.tile_pool(name="psq", bufs=2, space="PSUM") as psq_pool,
            tc.tile_pool(name="pso", bufs=4, space="PSUM") as pso_pool,
        ):
            # ---- load inputs; per-ct x tiles; serial sync triggers give a
            # natural stagger that keeps the early DMA rails on x ----
            wp0_sb = cpool.tile([P, CT * 256], BF, name="wp0", tag="wp0")
            x_sb = [cpool.tile([P, S], BF, name=f"x{ct}", tag=f"x{ct}")
                    for ct in range(CT)]
            wvv_sb = cpool.tile([P, CT * 512], BF, name="wvv", tag="wvv")
            wpr_sb = cpool.tile([P, CT * 768], BF, name="wpr", tag="wpr")
            wo_sb = cpool.tile([P, CT * C], BF, name="wosb", tag="wosb")
            wqv = wq_d.rearrange("(f p) e -> p f e", p=P)
            nc.sync.dma_start(
                wp0_sb.rearrange("p (f e) -> p f e", f=CT), wqv[:, :, 0:256]
            )
            for ct in range(CT):
                nc.sync.dma_start(x_sb[ct][:], x_d[ct * P:(ct + 1) * P, :])
            nc.sync.dma_start(
                wvv_sb.rearrange("p (f e) -> p f e", f=CT), wqv[:, :, 1024:1536]
            )
            nc.sync.dma_start(
                wpr_sb.rearrange("p (f e) -> p f e", f=CT), wqv[:, :, 256:1024]
            )
            nc.sync.dma_start(
                wo_sb.rearrange("p (f e) -> p f e", f=CT),
                wo_d.rearrange("(f p) e -> p f e", p=P),
            )

            def x_slice(ct, lo, hi):
                return x_sb[ct][:, lo:hi]

            def w_slice(col, ct):
                if col < 256:
                    return wp0_sb[:, ct * 256 + col:ct * 256 + col + 128]
                if col < 1024:
                    c = col - 256
                    return wpr_sb[:, ct * 768 + c:ct * 768 + c + 128]
                c = col - 1024
                return wvv_sb[:, ct * 512 + c:ct * 512 + c + 512]

            q_sb = [qkpool.tile([P, S], BF, name=f"q{p}", tag=f"q{p}")
                    for p in range(NP)]
            k_sb = [qkpool.tile([P, S], BF, name=f"k{p}", tag=f"k{p}")
                    for p in range(NP)]
            v_sb = [None] * MT
            oT_sb = [opool.tile([P, S], BF, name=f"o{ct}", tag=f"o{ct}")
                     for ct in range(CT)]

            def g_qkv(dst, col, nt, pool, copy_eng=None):
                ps = pool.tile([P, 512], F32, name="gq", tag=pool.name)
                for ct in range(CT):
                    nc.tensor.matmul(
                        ps[:],
                        w_slice(col, ct),
                        x_slice(ct, nt * 512, (nt + 1) * 512),
                        start=(ct == 0), stop=(ct == CT - 1),
                    )
                dst_sl = dst[:, nt * 512:(nt + 1) * 512]
                if copy_eng == "scalar":
                    nc.scalar.copy(dst_sl, ps[:])
                else:
                    nc.vector.tensor_copy(dst_sl, ps[:])

            def g_v(mt, pool):
                ps = pool.tile([P, 512], F32, name="gv", tag=pool.name)
                for ct in range(CT):
                    nc.tensor.matmul(
                        ps[:],
                        x_slice(ct, mt * P, (mt + 1) * P),
                        w_slice(1024, ct),
                        start=(ct == 0), stop=(ct == CT - 1),
                    )
                vt = vpool.tile([P, NH * (HD + 1)], BF, name=f"v{mt}", tag=f"v{mt}")
                vv = vt.rearrange("p (h e) -> p h e", e=HD + 1)
                nc.gpsimd.memset(vv[:, :, HD:HD + 1], 1.0)
                nc.vector.tensor_copy(
                    vv[:, :, 0:HD], ps.rearrange("p (h d) -> p h d", d=HD)
                )
                v_sb[mt] = vt

            def qk_groups(p):
                return [("q", p, 0), ("k", p, 0), ("q", p, 1), ("k", p, 1)]

            drains = {
                0: [("v", m, 0) for m in range(MT)] + qk_groups(1),
                1: qk_groups(2),
                2: qk_groups(3),
                3: [],
            }
            drain_quota = {
                0: [0, 1, 2, 2, 2, 2, 2, 1],
                1: [1, 1, 1, 1, 0, 0, 0, 0],
                2: [1, 1, 1, 1, 0, 0, 0, 0],
                3: [0] * 8,
            }

            def emit_drain(item):
                kind, a, nt = item
                if kind == "v":
                    g_v(a, psq_pool)
                elif kind == "q":
                    g_qkv(q_sb[a], a * 256, nt, psq_pool)
                else:
                    g_qkv(k_sb[a], a * 256 + 128, nt, psq_pool)

            def emit_qk_half(p, s, nt, ptiles):
                psq = psq_pool.tile([P, 1024], F32, name="psq", tag="psq")
                for hh in range(2):
                    nc.tensor.matmul(
                        psq[:, hh * 512:(hh + 1) * 512],
                        k_sb[p][hh * HD:(hh + 1) * HD, s * P:(s + 1) * P],
                        q_sb[p][hh * HD:(hh + 1) * HD, nt * 512:(nt + 1) * 512],
                        start=True, stop=True,
                    )
                pt = ppool.tile([P, 1024], BF, name="ptile", tag="ptile")
                nc.scalar.activation(
                    pt[:], psq[:], mybir.ActivationFunctionType.Exp,
                    scale=float(1.0 / np.sqrt(HD)),
                )
                ptiles[(s, nt)] = pt

            def emit_pv(pp, pso_t, ptiles, m):
                for idx, (hh, nt) in enumerate(DRAIN):
                    h = 2 * pp + hh
                    nc.tensor.matmul(
                        pso_t[idx][0:HD + 1, :],
                        v_sb[m][:, h * (HD + 1):(h + 1) * (HD + 1)],
                        ptiles[(m, nt)][:, hh * 512:(hh + 1) * 512],
                        start=(m == 0), stop=(m == MT - 1),
                    )

            def emit_chain_pre(pso_t):
                """recip+broadcast of the softmax denominators (cheap vector
                ops, no head-of-line risk)."""
                bc = []
                for idx in range(4):
                    r0 = mpool.tile([1, 512], F32, name="rr", tag="rr")
                    nc.vector.tensor_copy(r0[0:1, :], pso_t[idx][HD:HD + 1, :])
                    r1 = mpool.tile([1, 512], F32, name="ri", tag="ri")
                    nc.vector.reciprocal_approx_fast(r1[0:1, :], r0[0:1, :])
                    b0 = mpool.tile([HD, 512], F32, name="bc", tag="bc")
                    nc.gpsimd.partition_broadcast(b0[:], r1[0:1, :], channels=HD)
                    bc.append(b0)
                return bc

            def emit_chain_muls(pp, pso_t, bc):
                for idx, (hh, nt) in enumerate(DRAIN):
                    nc.vector.tensor_mul(
                        oT_sb[pp][hh * HD:(hh + 1) * HD, nt * 512:(nt + 1) * 512],
                        pso_t[idx][0:HD, :], bc[idx][:],
                    )

            def emit_tail_chain(pso_t, idxs):
                """rr+recip (vector), bcast (gpsimd), then muls — ordered to
                avoid vector head-of-line blocking."""
                bc = {}
                for idx in idxs:
                    r0 = mpool.tile([1, 512], F32, name="rr", tag="rr")
                    nc.vector.tensor_copy(r0[0:1, :], pso_t[idx][HD:HD + 1, :])
                    r1 = mpool.tile([1, 512], F32, name="ri", tag="ri")
                    nc.vector.reciprocal_approx_fast(r1[0:1, :], r0[0:1, :])
                    b0 = mpool.tile([HD, 512], F32, name="bc", tag="bc")
                    nc.gpsimd.partition_broadcast(b0[:], r1[0:1, :], channels=HD)
                    bc[idx] = b0
                for idx in idxs:
                    hh, nt = DRAIN[idx]
                    nc.vector.tensor_mul(
                        oT_sb[3][hh * HD:(hh + 1) * HD, nt * 512:(nt + 1) * 512],
                        pso_t[idx][0:HD, :], bc[idx][:],
                    )

            def proj_group(g, st, pool, copy_eng="scalar"):
                ps = pool.tile([P, 512], F32, name="op", tag=pool.name)
                for ct in range(CT):
                    nc.tensor.matmul(
                        ps[:],
                        wo_sb[:, ct * 512 + g * P:ct * 512 + (g + 1) * P],
                        oT_sb[ct][:, st * 512:(st + 1) * 512],
                        start=(ct == 0), stop=(ct == CT - 1),
                    )
                yt = ypool.tile([P, 512], BF, name="yt", tag="yt")
                if copy_eng == "scalar":
                    nc.scalar.activation(
                        yt[:], ps[:], mybir.ActivationFunctionType.Copy
                    )
                else:
                    nc.vector.tensor_copy(yt[:], ps[:])
                nc.sync.dma_start(
                    y_d[g * P:(g + 1) * P, st * 512:(st + 1) * 512], yt[:]
                )

            # ---- attention steps ----
            prev = None  # (p-1, pso_t, ptiles)
            for p in range(3):
                pso_t = [pso_pool.tile([P, 512], F32, name=f"pso{i}", tag="pso")
                         for i in range(4)]
                ptiles = {}
                dq = list(drains[p])
                quota = drain_quota[p]
                s_start = 0
                if p == 0:
                    # minimal pre-phase: just enough to start the exp chain
                    g_qkv(q_sb[0], 0, 0, pso_pool)
                    g_qkv(k_sb[0], 128, 0, pso_pool, copy_eng="scalar")
                    emit_qk_half(0, 0, 0, ptiles)
                    g_qkv(q_sb[0], 0, 1, pso_pool)
                    g_qkv(k_sb[0], 128, 1, pso_pool, copy_eng="scalar")
                    emit_qk_half(0, 0, 1, ptiles)
                    s_start = 1
                prev_bc = None
                for s in range(s_start, MT):
                    emit_qk_half(p, s, 0, ptiles)
                    emit_qk_half(p, s, 1, ptiles)
                    if prev is not None and s == 0:
                        emit_pv(prev[0], prev[1], prev[2], 7)
                    for m in PV_AT.get(s, ()):
                        emit_pv(p, pso_t, ptiles, m)
                    for _ in range(quota[s]):
                        if dq:
                            emit_drain(dq.pop(0))
                    if prev is not None and s == 0:
                        prev_bc = emit_chain_pre(prev[1])
                    if prev is not None and s == 3:
                        emit_chain_muls(prev[0], prev[1], prev_bc)
                prev = (p, pso_t, ptiles)

            # ---- last step (p=3): nt-split so the st0 half of the output
            # projection completes during the nt1 exp phase ----
            pso_t = [pso_pool.tile([P, 512], F32, name=f"pso{i}", tag="pso")
                     for i in range(4)]
            ptiles = {}

            def emit_pv3(m, idxs):
                for idx in idxs:
                    hh, nt = DRAIN[idx]
                    h = 6 + hh
                    nc.tensor.matmul(
                        pso_t[idx][0:HD + 1, :],
                        v_sb[m][:, h * (HD + 1):(h + 1) * (HD + 1)],
                        ptiles[(m, nt)][:, hh * 512:(hh + 1) * 512],
                        start=(m == 0), stop=(m == MT - 1),
                    )

            PV3_AT = {4: (0, 1), 5: (2, 3), 6: (4,)}  # (5,6) go to the nt1 phase
            prev_bc = None
            for s in range(MT):  # nt0 phase
                emit_qk_half(3, s, 0, ptiles)
                if s == 0:
                    emit_pv(prev[0], prev[1], prev[2], 7)
                for m in PV3_AT.get(s, ()):
                    emit_pv3(m, (0, 1))
                if s == 0:
                    prev_bc = emit_chain_pre(prev[1])
                if s == 3:
                    emit_chain_muls(prev[0], prev[1], prev_bc)
            for s in range(MT):  # nt1 phase
                emit_qk_half(3, s, 1, ptiles)
                # nt0 PV backlog rides behind the nt1 QKs so it never blocks
                # the exp chain
                if s == 0:
                    emit_pv3(5, (0, 1))
                if s == 1:
                    emit_pv3(6, (0, 1))
                    emit_pv3(7, (0, 1))
                for m in PV3_AT.get(s, ()):
                    emit_pv3(m, (2, 3))
                if s == 2:
                    emit_tail_chain(pso_t, (0, 1))
                if s == 3:
                    proj_group(0, 0, pso_pool, copy_eng="vector")
                    proj_group(1, 0, pso_pool, copy_eng="vector")
                if s == 4:
                    proj_group(2, 0, psq_pool, copy_eng="vector")
                if s == 5:
                    proj_group(3, 0, psq_pool, copy_eng="vector")
                if s == 6:
                    emit_pv3(5, (2, 3))
            # ---- tail: PV(3,6/7) nt1, chains idx2/3, st1 projection ----
            emit_pv3(6, (2, 3))
            emit_pv3(7, (2, 3))
            emit_tail_chain(pso_t, (2, 3))
            for g in range(CT):
                proj_group(g, 1, pso_pool)

    nc.compile()
    return nc


def get_program():
    if "nc" not in _cache:
        _cache["nc"] = build_program()
    return _cache["nc"]


_COL_ORDER = np.concatenate(
    [np.r_[p * 128:(p + 1) * 128, 512 + p * 128:512 + (p + 1) * 128]
     for p in range(NP)] + [np.r_[1024:1536]]
)


def kernel(x, w_qkv, w_out, b_out, _trace=False, _tmpdir=None):
    x = np.asarray(x, dtype=np.float32)
    w_qkv = np.asarray(w_qkv, dtype=np.float32)
    w_out = np.asarray(w_out, dtype=np.float32)
    b_out = np.asarray(b_out, dtype=np.float32)
    N = x.shape[0]

    xb = x.reshape(N, C, S).astype(ml_dtypes.bfloat16)
    wqT = np.ascontiguousarray(w_qkv.T[:, _COL_ORDER]).astype(ml_dtypes.bfloat16)
    woT = np.ascontiguousarray(w_out.T).astype(ml_dtypes.bfloat16)

    nc = get_program()
    in_maps = [
        {"x": np.ascontiguousarray(xb[n]), "wq": wqT, "wo": woT}
        for n in range(N)
    ]
    res = run_bass_kernel_spmd(
        nc, in_maps, core_ids=list(range(N)), trace=_trace, tmpdir=_tmpdir
    )
    y = np.stack([res.results[n]["y"] for n in range(N)]).astype(np.float32)
    y = y.reshape(N, C, 32, 32)
    y = y + b_out[None, :, None, None]
    if _trace:
        return y, res
    return y
